# revision 6
# baseline (speedup 1.0000x reference)
"""AnomalyAttention Trainium2 kernel (8 NeuronCores, data-parallel over batch).

Problem shapes: queries/keys/values [16,512,8,64] f32, sigma [16,512,8] f32,
attn_mask [16,512,512] bool (unused, mask_flag=False).

Returns (V, series, prior, sigma_full):
  V          [16,512,8,64]   = series @ values
  series     [16,8,512,512]  = softmax(Q K^T / 8)
  prior      [16,8,512,512]  = c/sig * exp(-(l-s)^2 / (2 sig^2))
  sigma_full [16,8,512,512]  = broadcast of transformed sigma

Sharding: batch 16 -> 2 per core across 8 cores. Everything per (b,h) is
independent; no collectives.
"""

import math
import sys

import numpy as np

sys.path.insert(0, "/opt/trn_rl_repo")

B, L, H, E = 16, 512, 8, 64
NCORES = 8
BPC = B // NCORES  # batches per core
LC = L // 128  # l-chunks of 128
HD = H * E  # 512
INV_SQRT_2PI = 1.0 / math.sqrt(2.0 * math.pi)

_CACHE = {}


def _build_nc():
    import concourse.bass as bass
    import concourse.mybir as mybir
    import concourse.tile as tile
    from concourse import bacc

    f32 = mybir.dt.float32
    nc = bacc.Bacc(
        "TRN2", target_bir_lowering=False, debug=False, num_devices=NCORES
    )

    q_dr = nc.declare_dram_parameter("queries", [BPC, L, HD], f32, isOutput=False)
    k_dr = nc.declare_dram_parameter("keys", [BPC, L, HD], f32, isOutput=False)
    v_dr = nc.declare_dram_parameter("values", [BPC, L, HD], f32, isOutput=False)
    # all constants packed into one [128, 2368] tensor:
    #   [0:2048]    negdist2 as (lc, s) blocks
    #   [2048:2176] identity
    #   [2176:2240] inv2s2   (col = b*H*LC + h*LC + lc)
    #   [2240:2304] logcs
    #   [2304:2368] sigt
    NSC = BPC * H * LC  # 64 scalar columns
    NCONST = 2048 + 128 + 3 * NSC
    const_dr = nc.declare_dram_parameter("consts", [128, NCONST], f32, isOutput=False)

    vo_dr = nc.declare_dram_parameter("v_out", [BPC, L, HD], f32, isOutput=True)
    ser_dr = nc.declare_dram_parameter("series", [BPC, H, L, L], f32, isOutput=True)
    pri_dr = nc.declare_dram_parameter("prior", [BPC, H, L, L], f32, isOutput=True)
    sig_dr = nc.declare_dram_parameter("sigma_full", [BPC, H, L, L], f32, isOutput=True)

    EXP = mybir.ActivationFunctionType.Exp

    with tile.TileContext(nc) as tc:
        with (
            tc.tile_pool(name="const", bufs=1) as cpool,
            tc.tile_pool(name="qkv", bufs=2) as qkvpool,
            tc.tile_pool(name="qt", bufs=2) as qtpool,
            tc.tile_pool(name="exp", bufs=2) as epool,
            tc.tile_pool(name="ser", bufs=2) as spool,
            tc.tile_pool(name="expt", bufs=2) as etpool,
            tc.tile_pool(name="pri", bufs=2) as ppool,
            tc.tile_pool(name="sg", bufs=2) as gpool,
            tc.tile_pool(name="va", bufs=2) as vapool,
            tc.tile_pool(name="rs", bufs=4) as rspool,
            tc.tile_pool(name="mm", bufs=3, space="PSUM") as mmpool,
            tc.tile_pool(name="vps", bufs=2, space="PSUM") as vpspool,
        ):
            # ---- constants (single DMA -> single wait for consumers) ----
            ct = cpool.tile([128, NCONST], f32, tag="consts")
            nc.sync.dma_start(out=ct[:], in_=const_dr[:])
            negd = ct[:, 0:2048].rearrange("p (lc s) -> p lc s", lc=LC)
            ident = ct[:, 2048:2176]
            inv2s2 = ct[:, 2176 : 2176 + NSC]
            logcs = ct[:, 2240 : 2240 + NSC]
            sigt = ct[:, 2304 : 2304 + NSC]

            for b in range(BPC):
                # ---- load natural-layout Q/K/V for this batch ----
                qn = qkvpool.tile([128, LC, HD], f32, tag="qn")
                nc.sync.dma_start(
                    out=qn[:], in_=q_dr[b].rearrange("(lc p) x -> p lc x", p=128)
                )
                kn = qkvpool.tile([128, LC, HD], f32, tag="kn")
                nc.sync.dma_start(
                    out=kn[:], in_=k_dr[b].rearrange("(lc p) x -> p lc x", p=128)
                )
                vn = qkvpool.tile([128, LC, HD], f32, tag="vn")
                nc.sync.dma_start(
                    out=vn[:], in_=v_dr[b].rearrange("(lc p) x -> p lc x", p=128)
                )

                # ---- transpose Q,K -> [he, l] layout (all heads at once) ----
                # qt[p, j, l] = Q[b, l, j*128 + p]  (he = j*128+p)
                qt = qtpool.tile([128, LC, L], f32, tag="qt")
                kt = qtpool.tile([128, LC, L], f32, tag="kt")
                for src, dst in ((qn, qt), (kn, kt)):
                    for lc in range(LC):
                        ps = vpspool.tile([128, 512], f32, tag="vps")
                        for j in range(4):
                            nc.tensor.transpose(
                                ps[:, j * 128 : (j + 1) * 128],
                                src[:, lc, j * 128 : (j + 1) * 128],
                                ident,
                            )
                        nc.vector.tensor_copy(
                            dst[:, :, lc * 128 : (lc + 1) * 128],
                            ps[:].rearrange("p (j l) -> p j l", j=4),
                        )

                va = vapool.tile([128, LC, HD], f32, tag="va")

                for h in range(H):
                    hp = (h % 2) * 64
                    j = h // 2
                    # lhsT/rhs views: Q^T,K^T for this head = [64 (e), 512 (l)]
                    qth = qt[hp : hp + 64, j, :]
                    kth = kt[hp : hp + 64, j, :]

                    # ---- scores = Q K^T / 8 ; exp + rowsums ----
                    ps_sc = [
                        mmpool.tile([128, 1024], f32, tag="mm", name="ps_sc")
                        for _ in range(2)
                    ]
                    for lc in range(LC):
                        nc.tensor.matmul(
                            ps_sc[lc // 2][:, (lc % 2) * 512 : (lc % 2) * 512 + 512],
                            qt[hp : hp + 64, j, lc * 128 : (lc + 1) * 128],
                            kth,
                        )
                    exp_sb = epool.tile([128, LC, L], f32, tag="exp")
                    rs = rspool.tile([128, LC], f32, tag="rs")
                    for lc in range(LC):
                        nc.scalar.activation(
                            exp_sb[:, lc, :],
                            ps_sc[lc // 2][:, (lc % 2) * 512 : (lc % 2) * 512 + 512],
                            EXP,
                            scale=0.125,
                            accum_out=rs[:, lc : lc + 1],
                        )
                    rc = rspool.tile([128, LC], f32, tag="rc")
                    nc.vector.reciprocal(rc[:], rs[:])

                    # ---- series = exp * (1/rowsum) ; DMA out ----
                    ser = spool.tile([128, LC, L], f32, tag="ser")
                    for lc in range(LC):
                        nc.vector.tensor_scalar_mul(
                            ser[:, lc, :], exp_sb[:, lc, :], rc[:, lc : lc + 1]
                        )
                    nc.sync.dma_start(
                        out=ser_dr[b, h].rearrange("(lc p) s -> p lc s", p=128),
                        in_=ser[:],
                    )

                    # ---- scores^T -> exp^T ----
                    ps_t = [
                        mmpool.tile([128, 1024], f32, tag="mm", name="ps_t")
                        for _ in range(2)
                    ]
                    for sc in range(LC):
                        nc.tensor.matmul(
                            ps_t[sc // 2][:, (sc % 2) * 512 : (sc % 2) * 512 + 512],
                            kt[hp : hp + 64, j, sc * 128 : (sc + 1) * 128],
                            qth,
                        )
                    expt = etpool.tile([128, LC, L], f32, tag="expt")
                    for half in range(2):
                        nc.scalar.activation(
                            expt[:, half * 2 : half * 2 + 2, :],
                            ps_t[half][:].rearrange("p (a s) -> p a s", a=2),
                            EXP,
                            scale=0.125,
                        )

                    # ---- V = (exp @ values) * (1/rowsum) ----
                    for lc in range(LC):
                        psv = vpspool.tile([128, 64], f32, tag="vps")
                        for sj in range(LC):
                            nc.tensor.matmul(
                                psv[:],
                                expt[:, sj, lc * 128 : (lc + 1) * 128],
                                vn[:, sj, h * 64 : (h + 1) * 64],
                                start=(sj == 0),
                                stop=(sj == LC - 1),
                            )
                        nc.vector.tensor_scalar_mul(
                            va[:, lc, h * 64 : (h + 1) * 64], psv[:], rc[:, lc : lc + 1]
                        )

                    # ---- prior = exp(negdist2 * inv2s2 + log(c/sig)) ----
                    c0 = b * (H * LC) + h * LC
                    pri = ppool.tile([128, LC, L], f32, tag="pri")
                    for lc in range(LC):
                        nc.scalar.activation(
                            pri[:, lc, :],
                            negd[:, lc, :],
                            EXP,
                            scale=inv2s2[:, c0 + lc : c0 + lc + 1],
                            bias=logcs[:, c0 + lc : c0 + lc + 1],
                        )
                    nc.sync.dma_start(
                        out=pri_dr[b, h].rearrange("(lc p) s -> p lc s", p=128),
                        in_=pri[:],
                    )

                    # ---- sigma_full = broadcast(sig) ----
                    sg = gpool.tile([128, LC, L], f32, tag="sg")
                    nc.gpsimd.tensor_copy(
                        sg[:], sigt[:, c0 : c0 + LC].to_broadcast([128, LC, L])
                    )
                    nc.sync.dma_start(
                        out=sig_dr[b, h].rearrange("(lc p) s -> p lc s", p=128),
                        in_=sg[:],
                    )

                # ---- V out for this batch ----
                nc.sync.dma_start(
                    out=vo_dr[b].rearrange("(lc p) x -> p lc x", p=128), in_=va[:]
                )

    nc.compile()
    return nc


def _host_consts(sigma):
    """Transform sigma on host; pack per-partition constant tables."""
    sig64 = sigma.astype(np.float64)  # [B, L, H]
    s = 1.0 / (1.0 + np.exp(-sig64 * 5.0)) + 1e-5
    s = np.power(3.0, s) - 1.0
    sigt = np.transpose(s, (0, 2, 1))  # [B, H, L]
    inv2s2 = 1.0 / (2.0 * sigt * sigt)
    logcs = np.log(INV_SQRT_2PI) - np.log(sigt)

    def pack(a):  # [B,H,L] -> per-core [128, BPC*H*LC], col = b*H*LC + h*LC + lc
        a = a.reshape(NCORES, BPC, H, LC, 128)
        return [
            np.ascontiguousarray(np.transpose(a[i], (3, 0, 1, 2)).reshape(128, -1))
            .astype(np.float32)
            for i in range(NCORES)
        ]

    return pack(inv2s2), pack(logcs), pack(sigt), sigt.astype(np.float32)


def kernel(queries, keys, values, sigma, attn_mask=None):
    from concourse.bass_utils import run_bass_kernel_spmd

    queries = np.ascontiguousarray(np.asarray(queries, np.float32))
    keys = np.ascontiguousarray(np.asarray(keys, np.float32))
    values = np.ascontiguousarray(np.asarray(values, np.float32))
    sigma = np.asarray(sigma, np.float32)

    if "nc" not in _CACHE:
        _CACHE["nc"] = _build_nc()
    nc = _CACHE["nc"]

    idx = np.arange(L, dtype=np.float32)
    d = idx[:, None] - idx[None, :]
    negdist2 = (-(d * d)).astype(np.float32)
    negd_packed = negdist2.reshape(4, 128, 512).transpose(1, 0, 2).reshape(128, 2048)
    ident = np.eye(128, dtype=np.float32)
    inv2s2_l, logcs_l, sigt_l, _ = _host_consts(sigma)

    q3 = queries.reshape(B, L, HD)
    k3 = keys.reshape(B, L, HD)
    v3 = values.reshape(B, L, HD)

    in_maps = []
    for i in range(NCORES):
        sl = slice(i * BPC, (i + 1) * BPC)
        in_maps.append(
            {
                "queries": np.ascontiguousarray(q3[sl]),
                "keys": np.ascontiguousarray(k3[sl]),
                "values": np.ascontiguousarray(v3[sl]),
                "consts": np.ascontiguousarray(
                    np.concatenate(
                        [negd_packed, ident, inv2s2_l[i], logcs_l[i], sigt_l[i]],
                        axis=1,
                    )
                ),
            }
        )

    trace = bool(_CACHE.get("trace", False))
    res = run_bass_kernel_spmd(nc, in_maps, core_ids=list(range(NCORES)), trace=trace)
    _CACHE["last_result"] = res

    V = np.concatenate([res.results[i]["v_out"] for i in range(NCORES)], axis=0)
    series = np.concatenate([res.results[i]["series"] for i in range(NCORES)], axis=0)
    prior = np.concatenate([res.results[i]["prior"] for i in range(NCORES)], axis=0)
    sigma_full = np.concatenate(
        [res.results[i]["sigma_full"] for i in range(NCORES)], axis=0
    )
    return (
        V.reshape(B, L, H, E),
        series,
        prior,
        sigma_full,
    )


# revision 15
# speedup vs baseline: 1.0063x; 1.0063x over previous
"""AnomalyAttention Trainium2 kernel (8 NeuronCores, data-parallel over batch).

Problem shapes: queries/keys/values [16,512,8,64] f32, sigma [16,512,8] f32,
attn_mask [16,512,512] bool (unused, mask_flag=False).

Returns (V, series, prior, sigma_full):
  V          [16,512,8,64]   = series @ values
  series     [16,8,512,512]  = softmax(Q K^T / 8)
  prior      [16,8,512,512]  = c/sig * exp(-(l-s)^2 / (2 sig^2))
  sigma_full [16,8,512,512]  = broadcast of transformed sigma

Sharding: batch 16 -> 2 per core across 8 cores. Everything per (b,h) is
independent; no collectives.

Layout trick: sequence rows are interleaved as l = 4p + j (partition p,
interleave j in 0..3) so each [128, 4, 512] staging tile maps to one
contiguous 8 KiB run per partition in DRAM -> maximal DMA descriptor size
on every big output. The QK^T / scores^T / V matmul chain is built in
this permuted order from the start (the QT column order encodes it), so
no extra data movement is needed anywhere.
"""

import math
import sys

import numpy as np

sys.path.insert(0, "/opt/trn_rl_repo")

B, L, H, E = 16, 512, 8, 64
NCORES = 8
BPC = B // NCORES  # batches per core
LC = L // 128  # l-chunks of 128
HD = H * E  # 512
INV_SQRT_2PI = 1.0 / math.sqrt(2.0 * math.pi)

_CACHE = {}


def _build_nc(variant="full", reps=1):
    import concourse.mybir as mybir
    import concourse.tile as tile
    from concourse import bacc

    f32 = mybir.dt.float32
    nc = bacc.Bacc(
        "TRN2", target_bir_lowering=False, debug=False, num_devices=NCORES
    )

    q_dr = nc.declare_dram_parameter("queries", [BPC, L, HD], f32, isOutput=False)
    k_dr = nc.declare_dram_parameter("keys", [BPC, L, HD], f32, isOutput=False)
    v_dr = nc.declare_dram_parameter("values", [BPC, L, HD], f32, isOutput=False)
    # all constants packed into one [128, 2368] tensor (row p, interleave j):
    #   [0:2048]    -dist^2 for rows l=4p+j  (negdist2.reshape(128, 2048))
    #   [2048:2176] identity
    #   [2176:2240] inv2s2   (col = b*H*LC + h*LC + j, row l=4p+j)
    #   [2240:2304] logcs
    #   [2304:2368] sigt
    NSC = BPC * H * LC  # 64 scalar columns
    NCONST = 2048 + 128 + 3 * NSC
    const_dr = nc.declare_dram_parameter("consts", [128, NCONST], f32, isOutput=False)

    vo_dr = nc.declare_dram_parameter("v_out", [BPC, L, HD], f32, isOutput=True)
    ser_dr = nc.declare_dram_parameter("series", [BPC, H, L, L], f32, isOutput=True)
    pri_dr = nc.declare_dram_parameter("prior", [BPC, H, L, L], f32, isOutput=True)
    sig_dr = nc.declare_dram_parameter("sigma_full", [BPC, H, L, L], f32, isOutput=True)

    EXP = mybir.ActivationFunctionType.Exp

    def interleaved(dr2d):
        # [512, N] dram view -> [128, 4, N] with row l = 4p + j
        return dr2d.rearrange("(p x) s -> p x s", p=128)

    with tile.TileContext(nc) as tc:
        with (
            tc.tile_pool(name="const", bufs=1) as cpool,
            tc.tile_pool(name="qkv", bufs=2) as qkvpool,
            tc.tile_pool(name="qt", bufs=2) as qtpool,
            tc.tile_pool(name="exp", bufs=2) as epool,
            tc.tile_pool(name="ser", bufs=2) as spool,
            tc.tile_pool(name="expt", bufs=2) as etpool,
            tc.tile_pool(name="pri", bufs=2) as ppool,
            tc.tile_pool(name="sg", bufs=2) as gpool,
            tc.tile_pool(name="va", bufs=2) as vapool,
            tc.tile_pool(name="rs", bufs=4) as rspool,
            tc.tile_pool(name="mm", bufs=3, space="PSUM") as mmpool,
            tc.tile_pool(name="vps", bufs=2, space="PSUM") as vpspool,
        ):
            # ---- constants (single DMA -> single wait for consumers) ----
            ct = cpool.tile([128, NCONST], f32, tag="consts")
            nc.sync.dma_start(out=ct[:], in_=const_dr[:])
            negd = ct[:, 0:2048].rearrange("p (j s) -> p j s", j=LC)
            ident = ct[:, 2048:2176]
            inv2s2 = ct[:, 2176 : 2176 + NSC]
            logcs = ct[:, 2240 : 2240 + NSC]
            sigt = ct[:, 2304 : 2304 + NSC]

            if variant == "noop":
                z = gpool.tile([128, 128], f32, tag="sg", name="z")
                nc.vector.tensor_copy(z[:], ct[:, 0:128])
                for dr in (vo_dr, ser_dr, pri_dr, sig_dr):
                    nc.sync.dma_start(
                        out=dr[:].rearrange("b x s -> (b x) s")[0:128, 0:128]
                        if dr is vo_dr
                        else dr[0, 0][0:128, 0:128],
                        in_=z[:],
                    )

            for b in range(BPC * reps):
                b = b % BPC
                if variant == "noop":
                    continue
                if variant == "sigonly":
                    for h in range(H):
                        c0 = b * (H * LC) + h * LC
                        sg = gpool.tile([128, LC, L], f32, tag="sg", name="sg")
                        nc.vector.tensor_copy(
                            sg[:], sigt[:, c0 : c0 + LC].to_broadcast([128, LC, L])
                        )
                        nc.sync.dma_start(out=interleaved(sig_dr[b, h]), in_=sg[:])
                    continue

                # ---- load natural-layout Q/K/V for this batch ----
                qn = qkvpool.tile([128, LC, HD], f32, tag="qn")
                nc.sync.dma_start(
                    out=qn[:], in_=q_dr[b].rearrange("(lc p) x -> p lc x", p=128)
                )
                kn = qkvpool.tile([128, LC, HD], f32, tag="kn")
                nc.sync.dma_start(
                    out=kn[:], in_=k_dr[b].rearrange("(lc p) x -> p lc x", p=128)
                )
                vn = qkvpool.tile([128, LC, HD], f32, tag="vn")
                nc.sync.dma_start(
                    out=vn[:], in_=v_dr[b].rearrange("(lc p) x -> p lc x", p=128)
                )

                # ---- transpose Q,K -> [he, l] with PERMUTED l-columns ----
                # qt[p, jj, c]: he = jj*128+p; column c = j*128 + pl holds
                # l = 4*pl + j. Source chunk lc, src pos q (l = lc*128 + q):
                # j = q % 4, pl = 32*lc + q//4.
                qt = qtpool.tile([128, LC, L], f32, tag="qt")
                kt = qtpool.tile([128, LC, L], f32, tag="kt")
                for src, dst, perm in ((qn, qt, True), (kn, kt, False)):
                    for lc in range(LC):
                        ps = vpspool.tile([128, 512], f32, tag="vps", name="ps")
                        for jj in range(4):
                            nc.tensor.transpose(
                                ps[:, jj * 128 : (jj + 1) * 128],
                                src[:, lc, jj * 128 : (jj + 1) * 128],
                                ident,
                            )
                        if perm:
                            # qt: l-columns interleaved. src pos q = 4a+r
                            # -> column r*128 + 32*lc + a
                            dst_v = dst[:].rearrange("p jj (r a) -> p jj r a", r=4)[
                                :, :, :, 32 * lc : 32 * lc + 32
                            ]
                            src_v = ps[:].rearrange(
                                "p (jj a r) -> p jj r a", jj=4, a=32
                            )
                            nc.vector.tensor_copy(dst_v, src_v)
                        else:
                            # kt: s-columns stay natural (block copy)
                            nc.vector.tensor_copy(
                                dst[:, :, lc * 128 : (lc + 1) * 128],
                                ps[:].rearrange("p (jj q) -> p jj q", jj=4),
                            )

                va = vapool.tile([128, LC, HD], f32, tag="va")

                for h in range(H):
                    hp = (h % 2) * 64
                    jh = h // 2
                    # Q^T,K^T for this head: [64 (e), 512]
                    qth = qt[hp : hp + 64, jh, :]
                    kth = kt[hp : hp + 64, jh, :]

                    # ---- scores = Q K^T / 8 ; exp + rowsums ----
                    ps_sc = [
                        mmpool.tile([128, 1024], f32, tag="mm", name="ps_sc")
                        for _ in range(2)
                    ]
                    for j in range(LC):
                        nc.tensor.matmul(
                            ps_sc[j // 2][:, (j % 2) * 512 : (j % 2) * 512 + 512],
                            qt[hp : hp + 64, jh, j * 128 : (j + 1) * 128],
                            kth,
                        )
                    exp_sb = epool.tile([128, LC, L], f32, tag="exp")
                    rs = rspool.tile([128, LC], f32, tag="rs")
                    for j in range(LC):
                        nc.scalar.activation(
                            exp_sb[:, j, :],
                            ps_sc[j // 2][:, (j % 2) * 512 : (j % 2) * 512 + 512],
                            EXP,
                            scale=0.125,
                            accum_out=rs[:, j : j + 1],
                        )
                    rc = rspool.tile([128, LC], f32, tag="rc")
                    nc.vector.reciprocal(rc[:], rs[:])

                    # ---- series = exp * (1/rowsum) ; DMA out (SWDGE) ----
                    ser = spool.tile([128, LC, L], f32, tag="ser")
                    for j in range(LC):
                        nc.vector.tensor_scalar_mul(
                            ser[:, j, :], exp_sb[:, j, :], rc[:, j : j + 1]
                        )
                    if variant == "full":
                        nc.sync.dma_start(out=interleaved(ser_dr[b, h]), in_=ser[:])

                    # ---- scores^T -> exp^T (l-columns permuted via qth) ----
                    ps_t = [
                        mmpool.tile([128, 1024], f32, tag="mm", name="ps_t")
                        for _ in range(2)
                    ]
                    for sc in range(LC):
                        nc.tensor.matmul(
                            ps_t[sc // 2][:, (sc % 2) * 512 : (sc % 2) * 512 + 512],
                            kt[hp : hp + 64, jh, sc * 128 : (sc + 1) * 128],
                            qth,
                        )
                    expt = etpool.tile([128, LC, L], f32, tag="expt")
                    for half in range(2):
                        nc.scalar.activation(
                            expt[:, half * 2 : half * 2 + 2, :],
                            ps_t[half][:].rearrange("p (a s) -> p a s", a=2),
                            EXP,
                            scale=0.125,
                        )

                    # ---- V = (exp @ values) * (1/rowsum) ----
                    for j in range(LC):
                        psv = vpspool.tile([128, 64], f32, tag="vps", name="psv")
                        for sj in range(LC):
                            nc.tensor.matmul(
                                psv[:],
                                expt[:, sj, j * 128 : (j + 1) * 128],
                                vn[:, sj, h * 64 : (h + 1) * 64],
                                start=(sj == 0),
                                stop=(sj == LC - 1),
                            )
                        nc.vector.tensor_scalar_mul(
                            va[:, j, h * 64 : (h + 1) * 64], psv[:], rc[:, j : j + 1]
                        )

                    # ---- prior = exp(negdist2 * inv2s2 + log(c/sig)) ----
                    c0 = b * (H * LC) + h * LC
                    pri = ppool.tile([128, LC, L], f32, tag="pri")
                    for j in range(LC):
                        nc.scalar.activation(
                            pri[:, j, :],
                            negd[:, j, :],
                            EXP,
                            scale=inv2s2[:, c0 + j : c0 + j + 1],
                            bias=logcs[:, c0 + j : c0 + j + 1],
                        )
                    if variant == "full":
                        nc.sync.dma_start(out=interleaved(pri_dr[b, h]), in_=pri[:])

                    # ---- sigma_full = broadcast(sig) (DVE), DMA on SP ----
                    if variant == "full":
                        sg = gpool.tile([128, LC, L], f32, tag="sg")
                        nc.vector.tensor_copy(
                            sg[:], sigt[:, c0 : c0 + LC].to_broadcast([128, LC, L])
                        )
                        nc.sync.dma_start(out=interleaved(sig_dr[b, h]), in_=sg[:])

                # ---- V out for this batch ----
                nc.sync.dma_start(out=interleaved(vo_dr[b]), in_=va[:])

    nc.compile()
    return nc


def _host_consts(sigma):
    """Transform sigma on host; pack per-partition constant tables.

    Row p / column (b,h,j) holds the value for l = 4p + j.
    """
    sig64 = sigma.astype(np.float64)  # [B, L, H]
    s = 1.0 / (1.0 + np.exp(-sig64 * 5.0)) + 1e-5
    s = np.power(3.0, s) - 1.0
    sigt = np.transpose(s, (0, 2, 1))  # [B, H, L]
    inv2s2 = 1.0 / (2.0 * sigt * sigt)
    logcs = np.log(INV_SQRT_2PI) - np.log(sigt)

    def pack(a):  # [B,H,L] -> per-core [128, BPC*H*LC], l = 4p + j
        a = a.reshape(NCORES, BPC, H, 128, LC)
        return [
            np.ascontiguousarray(
                np.transpose(a[i], (2, 0, 1, 3)).reshape(128, -1)
            ).astype(np.float32)
            for i in range(NCORES)
        ]

    return pack(inv2s2), pack(logcs), pack(sigt), sigt.astype(np.float32)


def _make_consts(inv2s2_i, logcs_i, sigt_i):
    idx = np.arange(L, dtype=np.float32)
    d = idx[:, None] - idx[None, :]
    negd_packed = (-(d * d)).astype(np.float32).reshape(128, 2048)  # l = 4p+j
    ident = np.eye(128, dtype=np.float32)
    return np.ascontiguousarray(
        np.concatenate([negd_packed, ident, inv2s2_i, logcs_i, sigt_i], axis=1)
    )


def kernel(queries, keys, values, sigma, attn_mask=None):
    from concourse.bass_utils import run_bass_kernel_spmd

    queries = np.ascontiguousarray(np.asarray(queries, np.float32))
    keys = np.ascontiguousarray(np.asarray(keys, np.float32))
    values = np.ascontiguousarray(np.asarray(values, np.float32))
    sigma = np.asarray(sigma, np.float32)

    if "nc" not in _CACHE:
        _CACHE["nc"] = _build_nc()
    nc = _CACHE["nc"]

    inv2s2_l, logcs_l, sigt_l, _ = _host_consts(sigma)

    q3 = queries.reshape(B, L, HD)
    k3 = keys.reshape(B, L, HD)
    v3 = values.reshape(B, L, HD)

    in_maps = []
    for i in range(NCORES):
        sl = slice(i * BPC, (i + 1) * BPC)
        in_maps.append(
            {
                "queries": np.ascontiguousarray(q3[sl]),
                "keys": np.ascontiguousarray(k3[sl]),
                "values": np.ascontiguousarray(v3[sl]),
                "consts": _make_consts(inv2s2_l[i], logcs_l[i], sigt_l[i]),
            }
        )

    res = run_bass_kernel_spmd(nc, in_maps, core_ids=list(range(NCORES)))
    _CACHE["last_result"] = res

    def gather(name):
        return np.concatenate([res.results[i][name] for i in range(NCORES)], axis=0)

    return (
        gather("v_out").reshape(B, L, H, E),
        gather("series"),
        gather("prior"),
        gather("sigma_full"),
    )
